# revision 67
# baseline (speedup 1.0000x reference)
"""ContextAwareSpanClassifier Trainium2 Bass kernel (v2).

Problem (hardcoded): B=4, S=2048, H=768, L=9, M=5 (window W=11).
  base_logits = x @ Wc + bc
  s = x . wa + ba ; windowed softmax over [t-5, t+5] (seq-edge masked)
  ctx[t] = sum_o attn[t,o] * x[t+o]
  h = gelu_erf(LN(cat(x,ctx) @ W1 + b1) * gamma + beta)
  out = 0.5*base_logits + 0.5*(h @ W2 + b2)

Sharding: data parallel over B*S = 8192 tokens -> 8 cores x 1024 tokens
(core c: batch c//2, seq half (c%2)*1024) with 5-token zero-padded halos.
Params replicated. ba shift cancels in softmax and is otherwise unused.

v2 design notes (vs v1):
  - Everything on the PE rides bf16 (host-cast): halves DMA bytes and
    doubles transpose rate; fp32r matmul at >=256 cols was already at
    bf16 rate so GEMM speed is unchanged, but the p-state model rewards
    a dense PE stream, so all PE work is ordered back-to-back and the
    LN sum matmuls are deferred one m-tile so PE never waits on ACT/DVE.
  - Input DMA consolidated: 2 const packs + 3 x chunks + 2 weight
    chunks + emask (HWDGE fixed cost ~630ns per DMA).
  - exp() runs on the [1, chunk] score rows BEFORE the s->column
    transposes; e_col comes out of PE transposes directly.
  - logits stay feature-major [9, 1024]; host transposes for free.
  - ACT table-load sequence per iteration: Exp, Sqrt(c0), Gelu(c0),
    Sqrt(c1), Gelu(c1); Identity/Copy/Square live in every table.
  - ones matmul stationary carries 1/H so PSUM sums are means directly.
"""

from contextlib import ExitStack

import numpy as np
import ml_dtypes

import concourse.bass as bass
import concourse.tile as tile
from concourse import bacc, mybir
from concourse.bass_utils import run_bass_kernel_spmd

F32 = mybir.dt.float32
BF16 = mybir.dt.bfloat16
AF = mybir.ActivationFunctionType
ALU = mybir.AluOpType

B, S, H = 4, 2048, 768
L, M = 9, 5
TOK = 1024             # tokens per core
NT = 8                 # 128-token output tiles per core
NJ = 9                 # x storage tiles (tile 8 has 10 valid rows)
FLAT = TOK + 2 * M     # 1034
FPAD = 1040
HC = H // 128          # 6
KC = 2 * H // 128      # 12
EPS = 1e-5
WCOL = H + L           # 777: w1s packs [W1 | Wc;W2] along columns

# const pack (bf16) column offsets
CB_ONES = 0        # [128,128] value 1/H
CB_ID = 128        # [128,128] identity
CB_BAND = 256      # [128,128] band mask
CB_CORN = 384      # [128,128] corner mask (rows 0..9)
CB_WA = 512        # [128,6] wa feature-major
CB_ONE1 = 518      # [128,1] value 1.0 (ecr replication source column)
CB_COLS = 519

# const pack (f32) column offsets
CF_B1 = 0          # [128,6]
CF_GAMMA = 6       # [128,6]
CF_BETA = 12       # [128,6]
CF_BIAS9 = 18      # rows 0..8 hold 0.5*(bc+b2)
CF_COLS = 19


def make_pools(tc, ctx):
    p = {}
    p["const"] = ctx.enter_context(tc.tile_pool(name="const", bufs=1))
    p["persist"] = ctx.enter_context(tc.tile_pool(name="persist", bufs=1))
    p["h"] = ctx.enter_context(tc.tile_pool(name="h", bufs=2))
    p["g"] = ctx.enter_context(tc.tile_pool(name="g", bufs=2))
    p["small"] = ctx.enter_context(tc.tile_pool(name="small", bufs=1))
    p["ln"] = ctx.enter_context(tc.tile_pool(name="ln", bufs=2))
    p["lt"] = ctx.enter_context(tc.tile_pool(name="lt", bufs=3))
    p["ps_tp"] = ctx.enter_context(tc.tile_pool(name="ps_tp", bufs=2, space="PSUM"))
    p["ps_mm"] = ctx.enter_context(tc.tile_pool(name="ps_mm", bufs=2, space="PSUM"))
    p["ps_st"] = ctx.enter_context(tc.tile_pool(name="ps_st", bufs=1, space="PSUM"))
    p["ps_sm"] = ctx.enter_context(tc.tile_pool(name="ps_sm", bufs=2, space="PSUM"))
    return p


def body(nc, tc, io, p):
    (x_d, emask_d, w1s_d, cb_d, cf_d, out_d) = io
    cpool, ppool = p["const"], p["persist"]
    hpool, gpool, spool = p["h"], p["g"], p["small"]
    lnpool, ltpool = p["ln"], p["lt"]
    ps_tp, ps_mm, ps_st, ps_sm = p["ps_tp"], p["ps_mm"], p["ps_st"], p["ps_sm"]

    # ---- DMAs. One ring (sync) carries x + weights in priority order so the
    # serial DMA device never starves the startup-critical x chunks; the ACT
    # ring carries the small consts and the per-chunk output stores (so the
    # next For_i iteration's x load can issue early on the sync ring).
    cb = cpool.tile([128, CB_COLS], BF16, tag="cb")
    nc.scalar.dma_start(out=cb, in_=cb_d)
    cf = cpool.tile([128, CF_COLS], F32, tag="cf")
    nc.scalar.dma_start(out=cf, in_=cf_d)

    x_view = x_d.rearrange("(j p) h -> p j h", p=128)
    xbf = ppool.tile([128, NJ, H], BF16, tag="xbf")
    w1s = cpool.tile([128, KC, WCOL], BF16, tag="w1s")
    w1s_view = w1s_d.rearrange("(k p) m -> p k m", p=128)
    nc.sync.dma_start(out=xbf[:, 0:2, :], in_=x_view[:, 0:2, :])
    nc.sync.dma_start(out=xbf[:, 2:5, :], in_=x_view[:, 2:5, :])
    nc.sync.dma_start(out=xbf[:, 5:9, :], in_=x_view[:, 5:9, :])
    nc.sync.dma_start(out=w1s[:, 0:6, :], in_=w1s_view[:, 0:6, :])
    nc.sync.dma_start(out=w1s[:, 6:12, :], in_=w1s_view[:, 6:12, :])
    emask_sb = cpool.tile([128, NJ], BF16, tag="emask")
    nc.sync.dma_start(out=emask_sb, in_=emask_d)

    ones_sb = cb[:, CB_ONES:CB_ONES + 128]          # value 1/H
    id_sb = cb[:, CB_ID:CB_ID + 128]
    mband_sb = cb[:, CB_BAND:CB_BAND + 128]
    mcorn_sb = cb[:, CB_CORN:CB_CORN + 128]
    wa_sb = cb[:, CB_WA:CB_WA + HC]

    eps_sb = cpool.tile([128, 1], F32, tag="eps")
    nc.vector.memset(eps_sb, EPS)
    idf = cpool.tile([1, 1], F32, tag="idf")
    nc.vector.memset(idf, 1.0)

    # ---- transposes: xbf token-major tiles -> xT feature-major ----
    xT = ppool.tile([128, HC, FPAD], BF16, tag="xT")

    def transpose_tile(j):
        rows = 128 if j < NJ - 1 else 10
        pt = ps_tp.tile([128, H], BF16, tag="tp")
        for hc in range(HC):
            nc.tensor.transpose(
                pt[:, hc * 128:hc * 128 + rows],
                xbf[:rows, j, hc * 128:(hc + 1) * 128],
                id_sb[:rows, :rows])
        dst = xT[:, :, 128 * j:128 * j + rows]
        src = pt.rearrange("p (c r) -> p c r", c=HC)[:, :, :rows]
        nc.vector.tensor_copy(out=dst, in_=src)

    def scores_chunk(e_row, c0, n):
        ps = ps_mm.tile([128, 512], F32, tag="mm")
        for hc in range(HC):
            nc.tensor.matmul(ps[:1, :n], wa_sb[:, hc:hc + 1],
                             xT[:, hc, c0:c0 + n],
                             start=(hc == 0), stop=(hc == HC - 1))
        nc.scalar.activation(out=e_row[:, c0:c0 + n], in_=ps[:1, :n],
                             func=AF.Exp)

    # ---- W1 GEMM pieces (emitted interleaved with the softmax phase).
    # Token chunks narrow toward the end so the last chunk's serial
    # LN->gelu->wstack chain (the kernel tail) runs on small tiles.
    CW = (512, 512)                          # chunk widths
    CO = (0, 512)                            # chunk col offsets
    NCH = len(CW)
    logitsT = ppool.tile([L, TOK], F32, tag="logitsT")
    ctxT = ppool.tile([128, HC, TOK], BF16, tag="ctxT")
    chunk_state = {}

    def w1_open(cch):
        chunk_state[cch] = dict(
            w=CW[cch], c0=CO[cch],
            h=hpool.tile([128, HC, 512], BF16, tag="h", name=f"h{cch}"),
            ps_s=ps_st.tile([128, 512], F32, tag="ss", name=f"ss{cch}"),
            ps_q=ps_st.tile([128, 512], F32, tag="sq", name=f"sq{cch}"),
            ph=[None] * HC, hsq=[None] * HC)

    def w1_k05(cch, m):
        st = chunk_state[cch]
        c0, w = st["c0"], st["w"]
        ph = ps_mm.tile([128, 512], F32, tag="mm")
        st["ph"][m] = ph
        for k in range(HC):
            nc.tensor.matmul(ph[:, :w], w1s[:, k, m * 128:(m + 1) * 128],
                             xT[:, k, M + c0:M + c0 + w],
                             start=(k == 0), stop=False)

    def w1_k611(cch, m):
        st = chunk_state[cch]
        c0, w = st["c0"], st["w"]
        ph = st["ph"][m]
        for k in range(HC, KC):
            nc.tensor.matmul(ph[:, :w], w1s[:, k, m * 128:(m + 1) * 128],
                             ctxT[:, k - HC, c0:c0 + w],
                             start=False, stop=(k == KC - 1))
        h_sb = st["h"]
        if m % 2 == 0:
            nc.scalar.activation(out=h_sb[:, m, :w], in_=ph[:, :w],
                                 func=AF.Identity,
                                 bias=cf[:, CF_B1 + m:CF_B1 + m + 1])
        else:
            nc.vector.tensor_scalar_add(out=h_sb[:, m, :w], in0=ph[:, :w],
                                        scalar1=cf[:, CF_B1 + m:CF_B1 + m + 1])
        hq = ltpool.tile([128, 512], BF16, tag="hsq")
        nc.gpsimd.tensor_mul(out=hq[:, :w], in0=h_sb[:, m, :w],
                             in1=h_sb[:, m, :w])
        st["hsq"][m] = hq

    def w1_sums(cch, m):
        st = chunk_state[cch]
        w = st["w"]
        nc.tensor.matmul(st["ps_s"][:, :w], ones_sb, st["h"][:, m, :w],
                         start=(m == 0), stop=(m == HC - 1))
        nc.tensor.matmul(st["ps_q"][:, :w], ones_sb, st["hsq"][m][:, :w],
                         start=(m == 0), stop=(m == HC - 1))

    def ln_stats_a(cch):
        st = chunk_state[cch]
        w = st["w"]
        musq = lnpool.tile([128, 512], F32, tag="musq")
        nc.scalar.activation(out=musq[:, :w], in_=st["ps_s"][:, :w],
                             func=AF.Square)
        var = lnpool.tile([128, 512], F32, tag="var")
        nc.vector.tensor_sub(out=var[:, :w], in0=st["ps_q"][:, :w],
                             in1=musq[:, :w])
        st["var"] = var
        # h - mu only needs the sums: all 6 tiles compute during sqrt below
        st["t"] = [None] * HC
        for m in range(HC):
            t = ltpool.tile([128, 512], BF16, tag=f"t{m}", bufs=2, name=f"t{m}")
            nc.vector.tensor_sub(out=t[:, :w], in0=st["h"][:, m, :w],
                                 in1=st["ps_s"][:, :w])
            st["t"][m] = t

    def ln_stats_b(cch):
        st = chunk_state[cch]
        w = st["w"]
        sd = lnpool.tile([128, 512], BF16, tag="sd")
        nc.scalar.activation(out=sd[:, :w], in_=st["var"][:, :w],
                             func=AF.Sqrt, bias=eps_sb)
        rstd = lnpool.tile([128, 512], BF16, tag="rstd")
        nc.vector.reciprocal(out=rstd[:, :w], in_=sd[:, :w])
        st["rstd"] = rstd
        st["o"] = [None] * HC
        st["gl"] = gpool.tile([128, HC, 512], BF16, tag="g", name=f"gl{cch}")

    def ln_div_m(cch, m):
        st = chunk_state[cch]
        w = st["w"]
        o = ltpool.tile([128, 512], BF16, tag=f"o{m}", bufs=2, name=f"o{m}")
        nc.vector.tensor_mul(out=o[:, :w], in0=st["t"][m][:, :w],
                             in1=st["rstd"][:, :w])
        st["o"][m] = o

    def gelu_m(cch, m):
        ln_div_m(cch, m)
        st = chunk_state[cch]
        w = st["w"]
        nc.scalar.activation(out=st["gl"][:, m, :w], in_=st["o"][m][:, :w],
                             func=AF.Gelu,
                             bias=cf[:, CF_BETA + m:CF_BETA + m + 1],
                             scale=cf[:, CF_GAMMA + m:CF_GAMMA + m + 1])

    def wstack_k05(cch):
        st = chunk_state[cch]
        c0, w = st["c0"], st["w"]
        pl = ps_mm.tile([128, 512], F32, tag="mm", name=f"pl{cch}")
        st["pl"] = pl
        for k in range(HC):
            nc.tensor.matmul(pl[:L, :w], w1s[:, k, H:H + L],
                             xT[:, k, M + c0:M + c0 + w],
                             start=(k == 0), stop=False)

    def wstack_gl(cch, m):
        st = chunk_state[cch]
        nc.tensor.matmul(st["pl"][:L, :st["w"]], w1s[:, HC + m, H:H + L],
                         st["gl"][:, m, :st["w"]],
                         start=False, stop=(m == HC - 1))

    def wstack_close(cch):
        st = chunk_state[cch]
        c0, w = st["c0"], st["w"]
        nc.scalar.activation(out=logitsT[:, c0:c0 + w], in_=st["pl"][:L, :w],
                             func=AF.Identity, bias=cf[:L, CF_BIAS9:CF_BIAS9 + 1],
                             scale=0.5)
        nc.scalar.dma_start(out=out_d[:, c0:c0 + w],
                            in_=logitsT[:, c0:c0 + w])

    # ---- phase 1: transposes chase the x DMAs; scores chase transposes;
    # then the x-only halves of the first two W1 m-tiles fill PE while the
    # softmax pipeline (e/D/A on ACT/DVE/Pool) runs.  (All three scores
    # chunks must be emitted before the open-ended W1 PSUM groups, or the
    # 2-slot "mm" rotation would deadlock the in-order PE queue.)
    e_row = spool.tile([1, FPAD], F32, tag="erow")
    pt_e = ps_sm.tile([128, 16], F32, tag="sm")
    e_col = spool.tile([128, NJ], F32, tag="ecol")
    onesb = cpool.tile([128, 128], BF16, tag="onesb")
    nc.gpsimd.memset(onesb, 1.0)
    ecr = [None] * NJ
    a_tiles = {}

    def e_transposes(j0, j1):
        for j in range(j0, j1):
            n = 128 if j < NJ - 1 else 10
            nc.tensor.transpose(pt_e[:n, j:j + 1],
                                e_row[:, 128 * j:128 * j + n], idf)

    def e_cols(j0, j1):
        # mask the freshly transposed e columns; replicate per tile on Pool
        nc.vector.tensor_mul(out=e_col[:, j0:j1], in0=pt_e[:, j0:j1],
                             in1=emask_sb[:, j0:j1])
        for j in range(j0, j1):
            t = spool.tile([128, 128], BF16, tag=f"ecr{j}", name=f"ecr{j}")
            nc.gpsimd.tensor_scalar_mul(out=t, in0=onesb,
                                        scalar1=e_col[:, j:j + 1])
            ecr[j] = t

    def softmax_group(jg):
        # D (window sums), R = 1/D, banded attention tiles A for one group
        pd = ps_sm.tile([128, 512], F32, tag="sm")
        for i in range(4):
            j = jg + i
            sl = slice(i * 128, (i + 1) * 128)
            nc.tensor.matmul(pd[:, sl], ecr[j], mband_sb, start=True, stop=False)
            nc.tensor.matmul(pd[:, sl], ecr[j + 1][:10, :], mcorn_sb[:10, :],
                             start=False, stop=True)
        r_rep = ltpool.tile([128, 512], BF16, tag="rrep")
        nc.vector.reciprocal(out=r_rep, in_=pd)
        for i in range(4):
            j = jg + i
            sl = slice(i * 128, (i + 1) * 128)
            am = spool.tile([128, 128], BF16, tag=f"am{jg}_{i}", name=f"am{jg}_{i}")
            nc.vector.scalar_tensor_tensor(
                out=am, in0=mband_sb, scalar=e_col[:, j:j + 1], in1=r_rep[:, sl],
                op0=ALU.mult, op1=ALU.mult)
            ac = spool.tile([16, 128], BF16, tag=f"ac{jg}_{i}", name=f"ac{jg}_{i}")
            nc.vector.scalar_tensor_tensor(
                out=ac[:10, :], in0=mcorn_sb[:10, :],
                scalar=e_col[:10, j + 1:j + 2], in1=r_rep[:10, sl],
                op0=ALU.mult, op1=ALU.mult)
            a_tiles[(jg, i)] = (am, ac)

    # Half-split softmax: group 0 (dest tiles 0-3, sources 0-4) only needs
    # x tiles 0-4 and score cols 0-639, so its whole D/R/A/ctx chain runs
    # while the second half's scores are still being produced.
    for j in range(5):
        transpose_tile(j)
    scores_chunk(e_row, 0, 512)
    scores_chunk(e_row, 512, 128)
    nc.vector.memset(pt_e, 0.0)
    e_transposes(0, 5)
    e_cols(0, 5)
    for j in range(5, NJ):
        transpose_tile(j)
    scores_chunk(e_row, 640, 384)
    scores_chunk(e_row, 1024, 10)
    softmax_group(0)
    e_transposes(5, NJ)
    e_cols(5, NJ)
    w1_open(0)

    def ctx_hc(jg, hc):
        pc = ps_sm.tile([128, 512], F32, tag="sm")
        for i in range(4):
            j = jg + i
            sl = slice(i * 128, (i + 1) * 128)
            am, ac = a_tiles[(jg, i)]
            nc.tensor.matmul(pc[:, sl], xbf[:, j, hc * 128:(hc + 1) * 128],
                             am, start=True, stop=False)
            nc.tensor.matmul(pc[:, sl],
                             xbf[:10, j + 1, hc * 128:(hc + 1) * 128],
                             ac[:10, :], start=False, stop=True)
        dst = ctxT[:, hc, 128 * jg:128 * jg + 512]
        if hc % 2:
            nc.scalar.copy(out=dst, in_=pc)
        else:
            nc.vector.tensor_copy(out=dst, in_=pc)

    for hc in range(HC):
        ctx_hc(0, hc)
    # x-only halves of the first two W1 m-tiles keep PE busy while ctx
    # group 0's PSUM->SBUF copies drain on DVE/ACT; ctx group 4 (only
    # needed by chunk 1) is drained into chunk 0's GEMM stream below.
    w1_k05(0, 0)
    w1_k05(0, 1)
    # the second half's D/R/A chain runs while chunk 0's GEMM occupies PE
    softmax_group(4)

    # ---- main GEMM stream with cross-chunk software pipelining: while
    # chunk c's W1 matmuls run on PE, chunk c-1's LN/gelu/wstack "postwork"
    # units are drained into the stream (each engine queue stays in
    # dependency-ready order).
    def postwork_units(cch):
        yield lambda: ln_stats_a(cch)
        yield lambda: ln_stats_b(cch)
        yield lambda: wstack_k05(cch)
        for m in range(HC):
            def u(m=m):
                gelu_m(cch, m)
                wstack_gl(cch, m)
            yield u

        def close():
            wstack_close(cch)
        yield close

    for cch in range(NCH):
        if cch > 0:
            w1_open(cch)
        if cch == 0:
            units = [lambda hc=hc: ctx_hc(4, hc) for hc in range(HC)]
        else:
            units = list(postwork_units(cch - 1))
        ui = 0
        for m in range(HC):
            if cch == 0 and m < 2:
                w1_k611(0, m)     # k05 halves were emitted in the ctx phase
            else:
                w1_k05(cch, m)
                w1_k611(cch, m)
            if m >= 1:
                w1_sums(cch, m - 1)
            for _ in range(2):
                if ui < len(units):
                    units[ui]()
                    ui += 1
        w1_sums(cch, HC - 1)
        while ui < len(units):
            units[ui]()
            ui += 1
    for unit in postwork_units(NCH - 1):
        unit()


def build(rep=1):
    nc = bacc.Bacc("TRN2", target_bir_lowering=False, debug=False, num_devices=8)

    x_d = nc.dram_tensor("x_loc", [NJ * 128, H], BF16, kind="ExternalInput").ap()
    emask_d = nc.dram_tensor("emask", [128, NJ], BF16, kind="ExternalInput").ap()
    w1s_d = nc.dram_tensor("w1s", [2 * H, WCOL], BF16, kind="ExternalInput").ap()
    cb_d = nc.dram_tensor("cb", [128, CB_COLS], BF16, kind="ExternalInput").ap()
    cf_d = nc.dram_tensor("cf", [128, CF_COLS], F32, kind="ExternalInput").ap()
    out_d = nc.dram_tensor("out_loc", [L, TOK], F32, kind="ExternalOutput").ap()

    io = (x_d, emask_d, w1s_d, cb_d, cf_d, out_d)

    with tile.TileContext(nc) as tc, ExitStack() as ctx, \
            nc.allow_low_precision(reason="tolerance 2e-2; bf16 by design"):
        p = make_pools(tc, ctx)
        if rep == 1:
            body(nc, tc, io, p)
        else:
            with tc.For_i(0, rep):
                body(nc, tc, io, p)
    nc.compile()
    return nc


def make_host_inputs(sequence_output, Wc, bc, wa, ba, W1, b1, gamma, beta, W2, b2):
    bf = ml_dtypes.bfloat16
    x = np.asarray(sequence_output, dtype=np.float32)

    w1s = np.concatenate(
        [np.asarray(W1, np.float32),
         np.concatenate([np.asarray(Wc, np.float32),
                         np.asarray(W2, np.float32)], axis=0)], axis=1
    ).astype(bf)                                             # [1536, 777]

    i_idx = np.arange(128)[:, None]
    j_idx = np.arange(128)[None, :]
    cbp = np.zeros((128, CB_COLS), np.float32)
    cbp[:, CB_ONES:CB_ONES + 128] = 1.0 / H
    cbp[:, CB_ID:CB_ID + 128] = np.eye(128)
    cbp[:, CB_BAND:CB_BAND + 128] = (j_idx <= i_idx) & (i_idx <= j_idx + 2 * M)
    corn = np.zeros((128, 128), np.float32)
    ii = np.arange(10)[:, None]
    corn[:10, :] = (j_idx >= 118 + ii)
    cbp[:, CB_CORN:CB_CORN + 128] = corn
    cbp[:, CB_WA:CB_WA + HC] = np.asarray(wa, np.float32).reshape(HC, 128).T
    cbp[:, CB_ONE1] = 1.0
    cbp = cbp.astype(bf)

    cfp = np.zeros((128, CF_COLS), np.float32)
    cfp[:, CF_B1:CF_B1 + HC] = np.asarray(b1, np.float32).reshape(HC, 128).T
    cfp[:, CF_GAMMA:CF_GAMMA + HC] = np.asarray(gamma, np.float32).reshape(HC, 128).T
    cfp[:, CF_BETA:CF_BETA + HC] = np.asarray(beta, np.float32).reshape(HC, 128).T
    cfp[:L, CF_BIAS9] = 0.5 * (np.asarray(bc, np.float32) + np.asarray(b2, np.float32))
    # ba: softmax is shift-invariant, and scores feed nothing else -> drop it.

    shared = {"w1s": w1s, "cb": cbp, "cf": cfp}

    in_maps = []
    for c in range(8):
        b, s0 = c // 2, TOK * (c % 2)
        x_loc = np.zeros((NJ * 128, H), np.float32)
        lo, hi = max(0, s0 - M), min(S, s0 + TOK + M)
        dst = lo - (s0 - M)
        x_loc[dst:dst + hi - lo] = x[b, lo:hi]
        f = np.arange(128)[:, None] + 128 * np.arange(NJ)[None, :]
        g = s0 + f - M
        emask = ((g >= 0) & (g < S) & (f < FLAT)).astype(bf)
        m = dict(shared)
        m["x_loc"] = x_loc.astype(bf)
        m["emask"] = emask
        in_maps.append(m)
    return in_maps


_cache = {}


def kernel(**inputs):
    if "nc" not in _cache:
        _cache["nc"] = build(rep=1)
    nc = _cache["nc"]
    in_maps = make_host_inputs(**inputs)
    res = run_bass_kernel_spmd(nc, in_maps, core_ids=list(range(8)))
    out = np.zeros((B, S, L), np.float32)
    for c in range(8):
        b, s0 = c // 2, TOK * (c % 2)
        out[b, s0:s0 + TOK] = np.asarray(res.results[c]["out_loc"]).T
    return out
